# revision 28
# baseline (speedup 1.0000x reference)
"""Multi-head attention kernel for 8 TRN2 NeuronCores.

Problem: B=4, S=2048, D=1024, H=16, DK=DV=64 multi-head attention with a
0/1 mask, f32 reference.

Sharding: 8 cores = 4 batches x 2 head-groups (8 heads each). Each core
computes, for its (batch, head-group): Q/K/V projections, masked softmax
attention, and a PARTIAL output projection (its heads' slice of Wo). The
two partials per batch are summed on the host (the tensor-parallel
all-reduce of the sharding hint, done host-side since full inputs/outputs
pass through the host anyway).

Device compute in bf16 with f32 PSUM accumulation:
 - Activations are pre-transposed on host: xq/xk/xv = x[b].T  [D, S].
 - Q^T,K^T computed as [j, s] (head pairs packed across 128 partitions);
   Q pre-scaled by 1/sqrt(DK).
 - Scores computed TRANSPOSED: S^T[t, s] = sum_j K^T[j,t] Q^T[j,s], so
   exp(S^T) directly feeds the attn@V matmul as the moving operand.
 - Softmax without max-subtraction (scores ~N(0,1); validated range).
   Mask applied multiplicatively after exp: P = exp(S^T) * maskT.
 - attn@V: lhsT = [V | ones] per head (65 cols) -> O^T rows 0..63 plus
   the softmax denominator (rowsum) in row 64, free on the PE.
 - t-blocks processed in pairs through a 2-bank PSUM tile (3 buffers):
   one exp and one mask multiply per pair, keeping PE runs uniform and
   per-op overheads amortized.
 - Normalization: rowsums DMA'd to partitions 0..7, reciprocal there,
   bounced through DRAM and broadcast across partitions by DMA; one
   elementwise multiply on the packed concat^T.
 - Output projection: packed head-pairs (k=128), result DMA'd straight
   from PSUM to DRAM.
"""

import numpy as np
import ml_dtypes
from contextlib import ExitStack

import concourse.bass as bass
import concourse.mybir as mybir
import concourse.tile as tile
from concourse import bacc
import concourse.bass_utils as bass_utils

P = 128
S = 2048          # sequence length
D = 1024          # model dim
HG = 8            # heads per core
DK = 64           # head dim
JW = HG * DK      # 512: packed projection width per core
DO = D // P       # 8 d-outer subtiles
NT = S // P       # 16 t-blocks
SC = 4            # s-chunks
SCW = S // SC     # 512 chunk width
NPAIR = HG // 2   # 4 head pairs
VW = DK + 1       # 65: V columns + ones column

bf16 = mybir.dt.bfloat16
f32 = mybir.dt.float32
AF = mybir.ActivationFunctionType
ALU = mybir.AluOpType


def _build():
    nc = bacc.Bacc("TRN2", target_bir_lowering=False, debug=False, num_devices=8)

    xq = nc.dram_tensor("xq", [D, S], bf16, kind="ExternalInput").ap()
    xk = nc.dram_tensor("xk", [D, S], bf16, kind="ExternalInput").ap()
    xv = nc.dram_tensor("xv", [D, S], bf16, kind="ExternalInput").ap()
    mt = nc.dram_tensor("mt", [S, S], bf16, kind="ExternalInput").ap()
    wq = nc.dram_tensor("wq", [D, JW], bf16, kind="ExternalInput").ap()
    wk = nc.dram_tensor("wk", [D, JW], bf16, kind="ExternalInput").ap()
    wv = nc.dram_tensor("wv", [D, JW], bf16, kind="ExternalInput").ap()
    wo = nc.dram_tensor("wo", [JW, D], bf16, kind="ExternalInput").ap()
    out = nc.dram_tensor("out", [S, D], f32, kind="ExternalOutput").ap()
    rscr = nc.dram_tensor("rscr", [SC, HG, SCW], bf16, kind="Internal").ap()
    rsum = nc.dram_tensor("rsum", [SC, HG, SCW], f32, kind="Internal").ap()

    with tile.TileContext(nc) as tc:
        with ExitStack() as ctx:
            consts = ctx.enter_context(tc.tile_pool(name="consts", bufs=1))
            stream = ctx.enter_context(tc.tile_pool(name="stream", bufs=6))
            mpool = ctx.enter_context(tc.tile_pool(name="mask", bufs=2))
            ppool = ctx.enter_context(tc.tile_pool(name="pp", bufs=4))
            epool = ctx.enter_context(tc.tile_pool(name="ep", bufs=3))
            rpool = ctx.enter_context(tc.tile_pool(name="rp", bufs=1))
            psA = ctx.enter_context(tc.tile_pool(name="psA", bufs=2, space="PSUM"))
            psO = ctx.enter_context(tc.tile_pool(name="psO", bufs=2, space="PSUM"))
            psP = ctx.enter_context(tc.tile_pool(name="psP", bufs=2, space="PSUM"))

            # ---- weights ----
            wq_sb = consts.tile([P, DO, JW], bf16, tag="wq")
            nc.sync.dma_start(wq_sb[:], wq.rearrange("(o p) j -> p o j", p=P))
            wk_sb = consts.tile([P, DO, JW], bf16, tag="wk")
            nc.sync.dma_start(wk_sb[:], wk.rearrange("(o p) j -> p o j", p=P))
            wv_sb = consts.tile([P, DO, JW], bf16, tag="wv")
            nc.sync.dma_start(wv_sb[:], wv.rearrange("(o p) j -> p o j", p=P))
            wo_sb = consts.tile([P, JW // P, D], bf16, tag="wo")
            nc.sync.dma_start(wo_sb[:], wo.rearrange("(o p) d -> p o d", p=P))

            # ---- persistent activations ----
            # QTz: per-head Q^T with the OTHER parity's 64 partitions zeroed, so
            # scores matmuls can contract over the full 128 partitions of KT
            # (uniform tile geometry on the PE; the off-parity K rows hit zeros).
            QTz = consts.tile([P, HG, S], bf16, tag="qt")
            KT = consts.tile([P, NPAIR, S], bf16, tag="kt")
            V = consts.tile([P, NT, HG * VW], bf16, tag="v")  # [t_in, t_out, 65h + (v|ones)]
            CT = consts.tile([P, NPAIR, S], bf16, tag="ct")   # concat^T, normalized in place

            nc.vector.memset(QTz[:], 0.0)
            for h in range(HG):
                nc.vector.memset(V[:, :, h * VW + DK : h * VW + DK + 1], 1.0)

            # ---- Q/K projections: dst[jo*128+m, s] = sum_d w[d, jo*128+m] x[d, s]
            for xin, wsb, which, scale in ((xq, wq_sb, "q", 1.0 / 8.0), (xk, wk_sb, "k", 1.0)):
                for st in range(SC):
                    pa = psA.tile([P, 2, SCW], f32, tag="s2", name="pa")
                    pb0 = psP.tile([P, SCW], f32, tag="po", name="pb0")
                    pb1 = psP.tile([P, SCW], f32, tag="po", name="pb1")
                    tgts = [pa[:, 0, :], pa[:, 1, :], pb0[:], pb1[:]]
                    for do in range(DO):
                        xt = stream.tile([P, SCW], bf16, tag="xt")
                        dma_eng = nc.sync if do % 2 == 0 else nc.gpsimd
                        dma_eng.dma_start(
                            xt[:], xin[do * P : (do + 1) * P, st * SCW : (st + 1) * SCW]
                        )
                        for jo in range(NPAIR):
                            nc.tensor.matmul(
                                tgts[jo],
                                lhsT=wsb[:, do, jo * P : (jo + 1) * P],
                                rhs=xt[:],
                                start=(do == 0),
                                stop=(do == DO - 1),
                            )
                    ssl = slice(st * SCW, (st + 1) * SCW)
                    if which == "q":
                        for jo in range(NPAIR):
                            # scalar engine for one parity, vector for the other
                            nc.scalar.activation(
                                QTz[0:64, 2 * jo, ssl], tgts[jo][0:64, :],
                                AF.Copy, scale=scale,
                            )
                            nc.vector.tensor_scalar_mul(
                                QTz[64:128, 2 * jo + 1, ssl], tgts[jo][64:128, :], scale
                            )
                    else:
                        nc.scalar.activation(KT[:, 0:2, ssl], pa[:], AF.Copy, scale=scale)
                        nc.vector.tensor_copy(KT[:, 2:3, ssl], pb0[:, None, :])
                        nc.vector.tensor_copy(KT[:, 3:4, ssl], pb1[:, None, :])

            # ---- V projection: V[t, v] = sum_d x[d, t]^T w[d, v], strided into [V|ones] slots
            Vv = V.rearrange("p t (h c) -> p t h c", h=HG)
            for tbq in range(NT // 4):
                pva = psA.tile([P, 2, SCW], f32, tag="s2", name="pva")
                pvb0 = psP.tile([P, SCW], f32, tag="po", name="pvb0")
                pvb1 = psP.tile([P, SCW], f32, tag="po", name="pvb1")
                vtgts = [pva[:, 0, :], pva[:, 1, :], pvb0[:], pvb1[:]]
                for do in range(DO):
                    xt = stream.tile([P, SCW], bf16, tag="xt")
                    dma_eng = nc.sync if do % 2 == 0 else nc.gpsimd
                    dma_eng.dma_start(
                        xt[:], xv[do * P : (do + 1) * P, tbq * SCW : (tbq + 1) * SCW]
                    )
                    for i in range(4):
                        nc.tensor.matmul(
                            vtgts[i],
                            lhsT=xt[:, i * P : (i + 1) * P],
                            rhs=wv_sb[:, do, :],
                            start=(do == 0), stop=(do == DO - 1),
                        )
                for i in range(4):
                    eng = nc.vector.tensor_copy if i % 2 == 0 else nc.scalar.activation
                    src = vtgts[i].rearrange("p (h c) -> p h c", h=HG)
                    if i % 2 == 0:
                        nc.vector.tensor_copy(Vv[:, 4 * tbq + i, :, 0:DK], src)
                    else:
                        nc.scalar.activation(Vv[:, 4 * tbq + i, :, 0:DK], src, AF.Copy)

            # ---- attention ----
            mtr = mt.rearrange("(to p) s -> p to s", p=P)

            def load_mask(sc):
                mk = mpool.tile([P, NT, SCW], bf16, tag="mk", name=f"mk{sc}")
                nc.gpsimd.dma_start(mk[:], mtr[:, :, sc * SCW : (sc + 1) * SCW])
                return mk

            mk_tiles = {0: load_mask(0)}
            for sc in range(SC):
                mk = mk_tiles.pop(sc)
                if sc + 1 < SC:
                    mk_tiles[sc + 1] = load_mask(sc + 1)
                rs = rpool.tile([P, HG, SCW], f32, tag="rs")  # rowsum staging @ partition 64
                R2 = rpool.tile([P, SCW // 8], f32, tag="R2")  # rowsums spread over 64 partitions

                for h in range(HG):
                    o = h // 2
                    Ops = psO.tile([P, SCW], f32, tag="o", name="Ops")
                    for g in range(NT // 2):
                        S2 = psA.tile([P, 2, SCW], f32, tag="s2", name="S2")
                        for i in range(2):
                            tb = 2 * g + i
                            nc.tensor.matmul(
                                S2[:, i, :],
                                lhsT=KT[:, o, tb * P : (tb + 1) * P],
                                rhs=QTz[:, h, sc * SCW : (sc + 1) * SCW],
                                start=True, stop=True,
                            )
                        Pt = ppool.tile([P, 2, SCW], bf16, tag="pt")
                        nc.scalar.activation(Pt[:], S2[:], AF.Exp)
                        nc.vector.tensor_tensor(
                            Pt[:], Pt[:], mk[:, 2 * g : 2 * g + 2, :], ALU.mult
                        )
                        for i in range(2):
                            tb = 2 * g + i
                            nc.tensor.matmul(
                                Ops[0:VW, :],
                                lhsT=V[:, tb, h * VW : (h + 1) * VW],
                                rhs=Pt[:, i, :],
                                start=(tb == 0), stop=(tb == NT - 1),
                            )
                    # evacuate rowsum + O^T
                    nc.vector.tensor_copy(rs[64:65, h, :], Ops[64:65, :])
                    if h % 2 == 0:
                        nc.vector.tensor_copy(
                            CT[0:64, o, sc * SCW : (sc + 1) * SCW], Ops[0:64, :]
                        )
                    else:
                        ob = epool.tile([64, SCW], bf16, tag="ob")
                        nc.vector.tensor_copy(ob[:], Ops[0:64, :])
                        nc.gpsimd.dma_start(
                            CT[64:128, o, sc * SCW : (sc + 1) * SCW], ob[:]
                        )

                # ---- normalization factors ----
                # bounce rowsums through DRAM to spread them over 64 partitions
                nc.sync.dma_start(rsum[sc].unsqueeze(0), rs[64:65, :, :])
                nc.sync.dma_start(
                    R2[0:64, :], rsum[sc].rearrange("h (p s) -> (h p) s", p=8)
                )
                Rr = rpool.tile([P, SCW // 8], bf16, tag="Rr")
                with nc.allow_low_precision(reason="softmax denominators tolerate bf16"):
                    nc.vector.reciprocal(Rr[0:64, :], R2[0:64, :])
                nc.sync.dma_start(
                    rscr[sc].rearrange("h (p s) -> (h p) s", p=8), Rr[0:64, :]
                )
                Rf = rpool.tile([P, NPAIR, SCW], bf16, tag="Rf")
                for par in range(2):
                    eng = nc.sync if par == 0 else nc.gpsimd
                    eng.dma_start(
                        Rf[64 * par : 64 * par + 64, :, :],
                        rscr[sc].rearrange("(pr two) s -> two pr s", two=2)[par]
                        .unsqueeze(0).to_broadcast([64, NPAIR, SCW]),
                    )

                # ---- normalize + output projection, per s-block so the PE can
                # start projecting before the whole chunk is normalized ----
                for sb in range(SCW // P):
                    s0 = sc * SCW + sb * P
                    nc.vector.tensor_tensor(
                        CT[:, :, s0 : s0 + P],
                        CT[:, :, s0 : s0 + P],
                        Rf[:, :, sb * P : (sb + 1) * P], ALU.mult,
                    )
                    for dt in range(D // SCW):
                        po = psP.tile([P, SCW], f32, tag="po", name="po")
                        for o2 in range(NPAIR):
                            nc.tensor.matmul(
                                po[:],
                                lhsT=CT[:, o2, s0 : s0 + P],
                                rhs=wo_sb[:, o2, dt * SCW : (dt + 1) * SCW],
                                start=(o2 == 0), stop=(o2 == NPAIR - 1),
                            )
                        ot = epool.tile([P, SCW], f32, tag="ot")
                        nc.vector.tensor_copy(ot[:], po[:])
                        nc.sync.dma_start(
                            out[s0 : s0 + P, dt * SCW : (dt + 1) * SCW], ot[:]
                        )

    nc.compile()
    return nc


_NC = None


def _get_nc():
    global _NC
    if _NC is None:
        _NC = _build()
    return _NC


def kernel(queries, keys, values, mask, Wq, Wk, Wv, Wo):
    bf = ml_dtypes.bfloat16
    B = queries.shape[0]
    nc = _get_nc()

    xqT = [queries[b].T.astype(bf) for b in range(B)]
    xkT = [keys[b].T.astype(bf) for b in range(B)]
    xvT = [values[b].T.astype(bf) for b in range(B)]
    mtT = [(mask[b] != 0).T.astype(bf) for b in range(B)]
    wqg = [np.transpose(Wq[HG * g : HG * (g + 1)], (1, 0, 2)).reshape(D, JW).astype(bf)
           for g in range(2)]
    wkg = [np.transpose(Wk[HG * g : HG * (g + 1)], (1, 0, 2)).reshape(D, JW).astype(bf)
           for g in range(2)]
    wvg = [np.transpose(Wv[HG * g : HG * (g + 1)], (1, 0, 2)).reshape(D, JW).astype(bf)
           for g in range(2)]
    wog = [Wo[JW * g : JW * (g + 1), :].astype(bf) for g in range(2)]

    in_maps = []
    for c in range(8):
        b, g = c // 2, c % 2
        in_maps.append({
            "xq": xqT[b], "xk": xkT[b], "xv": xvT[b], "mt": mtT[b],
            "wq": wqg[g], "wk": wkg[g], "wv": wvg[g], "wo": wog[g],
        })

    res = bass_utils.run_bass_kernel_spmd(nc, in_maps, core_ids=list(range(8)))
    outs = [r["out"] for r in res.results]
    return np.stack([outs[2 * b] + outs[2 * b + 1] for b in range(B)]).astype(np.float32)


# revision 31
# speedup vs baseline: 1.0028x; 1.0028x over previous
"""Multi-head attention kernel for 8 TRN2 NeuronCores.

Problem: B=4, S=2048, D=1024, H=16, DK=DV=64 multi-head attention with a
0/1 mask, f32 reference.

Sharding: 8 cores = 4 batches x 2 head-groups (8 heads each). Each core
computes, for its (batch, head-group): Q/K/V projections, masked softmax
attention, and a PARTIAL output projection (its heads' slice of Wo). The
two partials per batch are summed on the host (the tensor-parallel
all-reduce of the sharding hint, done host-side since full inputs/outputs
pass through the host anyway).

Device compute in bf16 with f32 PSUM accumulation:
 - Activations are pre-transposed on host: xq/xk/xv = x[b].T  [D, S].
 - Q^T,K^T computed as [j, s] (head pairs packed across 128 partitions);
   Q pre-scaled by 1/sqrt(DK).
 - Scores computed TRANSPOSED: S^T[t, s] = sum_j K^T[j,t] Q^T[j,s], so
   exp(S^T) directly feeds the attn@V matmul as the moving operand.
 - Softmax without max-subtraction (scores ~N(0,1); validated range).
   Mask applied multiplicatively after exp: P = exp(S^T) * maskT.
 - attn@V: lhsT = [V | ones] per head (65 cols) -> O^T rows 0..63 plus
   the softmax denominator (rowsum) in row 64, free on the PE.
 - t-blocks processed in pairs through a 2-bank PSUM tile (3 buffers):
   one exp and one mask multiply per pair, keeping PE runs uniform and
   per-op overheads amortized.
 - Normalization: rowsums DMA'd to partitions 0..7, reciprocal there,
   bounced through DRAM and broadcast across partitions by DMA; one
   elementwise multiply on the packed concat^T.
 - Output projection: packed head-pairs (k=128), result DMA'd straight
   from PSUM to DRAM.
"""

import numpy as np
import ml_dtypes
from contextlib import ExitStack

import concourse.bass as bass
import concourse.mybir as mybir
import concourse.tile as tile
from concourse import bacc
import concourse.bass_utils as bass_utils

P = 128
S = 2048          # sequence length
D = 1024          # model dim
HG = 8            # heads per core
DK = 64           # head dim
JW = HG * DK      # 512: packed projection width per core
DO = D // P       # 8 d-outer subtiles
NT = S // P       # 16 t-blocks
SC = 4            # s-chunks
SCW = S // SC     # 512 chunk width
NPAIR = HG // 2   # 4 head pairs
VW = DK + 1       # 65: V columns + ones column

bf16 = mybir.dt.bfloat16
f32 = mybir.dt.float32
AF = mybir.ActivationFunctionType
ALU = mybir.AluOpType


def _build():
    nc = bacc.Bacc("TRN2", target_bir_lowering=False, debug=False, num_devices=8)

    xq = nc.dram_tensor("xq", [D, S], bf16, kind="ExternalInput").ap()
    xk = nc.dram_tensor("xk", [D, S], bf16, kind="ExternalInput").ap()
    xv = nc.dram_tensor("xv", [D, S], bf16, kind="ExternalInput").ap()
    mt = nc.dram_tensor("mt", [S, S], bf16, kind="ExternalInput").ap()
    wq = nc.dram_tensor("wq", [D, JW], bf16, kind="ExternalInput").ap()
    wk = nc.dram_tensor("wk", [D, JW], bf16, kind="ExternalInput").ap()
    wv = nc.dram_tensor("wv", [D, JW], bf16, kind="ExternalInput").ap()
    wo = nc.dram_tensor("wo", [JW, D], bf16, kind="ExternalInput").ap()
    out = nc.dram_tensor("out", [S, D], f32, kind="ExternalOutput").ap()
    rscr = nc.dram_tensor("rscr", [SC, HG, SCW], bf16, kind="Internal").ap()
    rsum = nc.dram_tensor("rsum", [SC, HG, SCW], f32, kind="Internal").ap()

    with tile.TileContext(nc) as tc:
        with ExitStack() as ctx:
            consts = ctx.enter_context(tc.tile_pool(name="consts", bufs=1))
            stream = ctx.enter_context(tc.tile_pool(name="stream", bufs=6))
            mpool = ctx.enter_context(tc.tile_pool(name="mask", bufs=2))
            ppool = ctx.enter_context(tc.tile_pool(name="pp", bufs=4))
            epool = ctx.enter_context(tc.tile_pool(name="ep", bufs=3))
            rpool = ctx.enter_context(tc.tile_pool(name="rp", bufs=1))
            psA = ctx.enter_context(tc.tile_pool(name="psA", bufs=2, space="PSUM"))
            psO = ctx.enter_context(tc.tile_pool(name="psO", bufs=2, space="PSUM"))
            psP = ctx.enter_context(tc.tile_pool(name="psP", bufs=2, space="PSUM"))

            # ---- weights ----
            wq_sb = consts.tile([P, DO, JW], bf16, tag="wq")
            nc.sync.dma_start(wq_sb[:], wq.rearrange("(o p) j -> p o j", p=P))
            wk_sb = consts.tile([P, DO, JW], bf16, tag="wk")
            nc.sync.dma_start(wk_sb[:], wk.rearrange("(o p) j -> p o j", p=P))
            wv_sb = consts.tile([P, DO, JW], bf16, tag="wv")
            nc.sync.dma_start(wv_sb[:], wv.rearrange("(o p) j -> p o j", p=P))
            wo_sb = consts.tile([P, JW // P, D], bf16, tag="wo")
            nc.sync.dma_start(wo_sb[:], wo.rearrange("(o p) d -> p o d", p=P))

            # ---- persistent activations ----
            # QTz: per-head Q^T with the OTHER parity's 64 partitions zeroed, so
            # scores matmuls can contract over the full 128 partitions of KT
            # (uniform tile geometry on the PE; the off-parity K rows hit zeros).
            # V is padded so each head's attn@V lhsT slice [65h : 65h+128] is a
            # full 128 columns (FWL-eligible, uniform PE tile geometry); the
            # junk columns beyond [V | ones] write PSUM rows 65..127 which are
            # never read.
            VWID = VW * (HG - 1) + P  # 583
            QTz = consts.tile([P, HG, S], bf16, tag="qt")
            KT = consts.tile([P, NPAIR, S], bf16, tag="kt")
            V = consts.tile([P, NT, VWID], bf16, tag="v")  # [t_in, t_out, 65h + (v|ones)]
            CT = consts.tile([P, NPAIR, S], bf16, tag="ct")   # concat^T, normalized in place

            nc.vector.memset(QTz[:], 0.0)
            nc.vector.memset(V[:, :, HG * VW : VWID], 0.0)
            for h in range(HG):
                nc.vector.memset(V[:, :, h * VW + DK : h * VW + DK + 1], 1.0)

            # ---- Q/K projections: dst[jo*128+m, s] = sum_d w[d, jo*128+m] x[d, s]
            for xin, wsb, which, scale in ((xq, wq_sb, "q", 1.0 / 8.0), (xk, wk_sb, "k", 1.0)):
                for st in range(SC):
                    pa = psA.tile([P, 2, SCW], f32, tag="s2", name="pa")
                    pb0 = psP.tile([P, SCW], f32, tag="po", name="pb0")
                    pb1 = psP.tile([P, SCW], f32, tag="po", name="pb1")
                    tgts = [pa[:, 0, :], pa[:, 1, :], pb0[:], pb1[:]]
                    for do in range(DO):
                        xt = stream.tile([P, SCW], bf16, tag="xt")
                        dma_eng = nc.sync if do % 2 == 0 else nc.gpsimd
                        dma_eng.dma_start(
                            xt[:], xin[do * P : (do + 1) * P, st * SCW : (st + 1) * SCW]
                        )
                        for jo in range(NPAIR):
                            nc.tensor.matmul(
                                tgts[jo],
                                lhsT=wsb[:, do, jo * P : (jo + 1) * P],
                                rhs=xt[:],
                                start=(do == 0),
                                stop=(do == DO - 1),
                            )
                    ssl = slice(st * SCW, (st + 1) * SCW)
                    if which == "q":
                        for jo in range(NPAIR):
                            # scalar engine for one parity, vector for the other
                            nc.scalar.activation(
                                QTz[0:64, 2 * jo, ssl], tgts[jo][0:64, :],
                                AF.Copy, scale=scale,
                            )
                            nc.vector.tensor_scalar_mul(
                                QTz[64:128, 2 * jo + 1, ssl], tgts[jo][64:128, :], scale
                            )
                    else:
                        nc.scalar.activation(KT[:, 0:2, ssl], pa[:], AF.Copy, scale=scale)
                        nc.vector.tensor_copy(KT[:, 2:3, ssl], pb0[:, None, :])
                        nc.vector.tensor_copy(KT[:, 3:4, ssl], pb1[:, None, :])

            # ---- V projection: V[t, v] = sum_d x[d, t]^T w[d, v], strided into [V|ones] slots
            Vv = V[:, :, 0 : HG * VW].rearrange("p t (h c) -> p t h c", h=HG)
            for tbq in range(NT // 4):
                pva = psA.tile([P, 2, SCW], f32, tag="s2", name="pva")
                pvb0 = psP.tile([P, SCW], f32, tag="po", name="pvb0")
                pvb1 = psP.tile([P, SCW], f32, tag="po", name="pvb1")
                vtgts = [pva[:, 0, :], pva[:, 1, :], pvb0[:], pvb1[:]]
                for do in range(DO):
                    xt = stream.tile([P, SCW], bf16, tag="xt")
                    dma_eng = nc.sync if do % 2 == 0 else nc.gpsimd
                    dma_eng.dma_start(
                        xt[:], xv[do * P : (do + 1) * P, tbq * SCW : (tbq + 1) * SCW]
                    )
                    for i in range(4):
                        nc.tensor.matmul(
                            vtgts[i],
                            lhsT=xt[:, i * P : (i + 1) * P],
                            rhs=wv_sb[:, do, :],
                            start=(do == 0), stop=(do == DO - 1),
                        )
                for i in range(4):
                    eng = nc.vector.tensor_copy if i % 2 == 0 else nc.scalar.activation
                    src = vtgts[i].rearrange("p (h c) -> p h c", h=HG)
                    if i % 2 == 0:
                        nc.vector.tensor_copy(Vv[:, 4 * tbq + i, :, 0:DK], src)
                    else:
                        nc.scalar.activation(Vv[:, 4 * tbq + i, :, 0:DK], src, AF.Copy)

            # ---- attention ----
            mtr = mt.rearrange("(to p) s -> p to s", p=P)

            def load_mask(sc):
                mk = mpool.tile([P, NT, SCW], bf16, tag="mk", name=f"mk{sc}")
                nc.gpsimd.dma_start(mk[:], mtr[:, :, sc * SCW : (sc + 1) * SCW])
                return mk

            mk_tiles = {0: load_mask(0)}
            for sc in range(SC):
                mk = mk_tiles.pop(sc)
                if sc + 1 < SC:
                    mk_tiles[sc + 1] = load_mask(sc + 1)
                rs = rpool.tile([P, HG, SCW], f32, tag="rs")  # rowsum staging @ partition 64
                R2 = rpool.tile([P, SCW // 8], f32, tag="R2")  # rowsums spread over 64 partitions

                for h in range(HG):
                    o = h // 2
                    Ops = psO.tile([P, SCW], f32, tag="o", name="Ops")
                    for g in range(NT // 2):
                        S2 = psA.tile([P, 2, SCW], f32, tag="s2", name="S2")
                        for i in range(2):
                            tb = 2 * g + i
                            nc.tensor.matmul(
                                S2[:, i, :],
                                lhsT=KT[:, o, tb * P : (tb + 1) * P],
                                rhs=QTz[:, h, sc * SCW : (sc + 1) * SCW],
                                start=True, stop=True,
                            )
                        Pt = ppool.tile([P, 2, SCW], bf16, tag="pt")
                        nc.scalar.activation(Pt[:], S2[:], AF.Exp)
                        nc.vector.tensor_tensor(
                            Pt[:], Pt[:], mk[:, 2 * g : 2 * g + 2, :], ALU.mult
                        )
                        for i in range(2):
                            tb = 2 * g + i
                            nc.tensor.matmul(
                                Ops[:],
                                lhsT=V[:, tb, h * VW : h * VW + P],
                                rhs=Pt[:, i, :],
                                start=(tb == 0), stop=(tb == NT - 1),
                            )
                    # evacuate rowsum + O^T
                    nc.vector.tensor_copy(rs[64:65, h, :], Ops[64:65, :])
                    if h % 2 == 0:
                        nc.vector.tensor_copy(
                            CT[0:64, o, sc * SCW : (sc + 1) * SCW], Ops[0:64, :]
                        )
                    else:
                        ob = epool.tile([64, SCW], bf16, tag="ob")
                        nc.vector.tensor_copy(ob[:], Ops[0:64, :])
                        nc.gpsimd.dma_start(
                            CT[64:128, o, sc * SCW : (sc + 1) * SCW], ob[:]
                        )

                # ---- normalization factors ----
                # bounce rowsums through DRAM to spread them over 64 partitions
                nc.sync.dma_start(rsum[sc].unsqueeze(0), rs[64:65, :, :])
                nc.sync.dma_start(
                    R2[0:64, :], rsum[sc].rearrange("h (p s) -> (h p) s", p=8)
                )
                Rr = rpool.tile([P, SCW // 8], bf16, tag="Rr")
                with nc.allow_low_precision(reason="softmax denominators tolerate bf16"):
                    nc.vector.reciprocal(Rr[0:64, :], R2[0:64, :])
                nc.sync.dma_start(
                    rscr[sc].rearrange("h (p s) -> (h p) s", p=8), Rr[0:64, :]
                )
                Rf = rpool.tile([P, NPAIR, SCW], bf16, tag="Rf")
                for par in range(2):
                    eng = nc.sync if par == 0 else nc.gpsimd
                    eng.dma_start(
                        Rf[64 * par : 64 * par + 64, :, :],
                        rscr[sc].rearrange("(pr two) s -> two pr s", two=2)[par]
                        .unsqueeze(0).to_broadcast([64, NPAIR, SCW]),
                    )

                # ---- normalize + output projection, per s-block so the PE can
                # start projecting before the whole chunk is normalized ----
                for sb in range(SCW // P):
                    s0 = sc * SCW + sb * P
                    nc.vector.tensor_tensor(
                        CT[:, :, s0 : s0 + P],
                        CT[:, :, s0 : s0 + P],
                        Rf[:, :, sb * P : (sb + 1) * P], ALU.mult,
                    )
                    for dt in range(D // SCW):
                        po = psP.tile([P, SCW], f32, tag="po", name="po")
                        for o2 in range(NPAIR):
                            nc.tensor.matmul(
                                po[:],
                                lhsT=CT[:, o2, s0 : s0 + P],
                                rhs=wo_sb[:, o2, dt * SCW : (dt + 1) * SCW],
                                start=(o2 == 0), stop=(o2 == NPAIR - 1),
                            )
                        ot = epool.tile([P, SCW], f32, tag="ot")
                        nc.vector.tensor_copy(ot[:], po[:])
                        nc.sync.dma_start(
                            out[s0 : s0 + P, dt * SCW : (dt + 1) * SCW], ot[:]
                        )

    nc.compile()
    return nc


_NC = None


def _get_nc():
    global _NC
    if _NC is None:
        _NC = _build()
    return _NC


def kernel(queries, keys, values, mask, Wq, Wk, Wv, Wo):
    bf = ml_dtypes.bfloat16
    B = queries.shape[0]
    nc = _get_nc()

    xqT = [queries[b].T.astype(bf) for b in range(B)]
    xkT = [keys[b].T.astype(bf) for b in range(B)]
    xvT = [values[b].T.astype(bf) for b in range(B)]
    mtT = [(mask[b] != 0).T.astype(bf) for b in range(B)]
    wqg = [np.transpose(Wq[HG * g : HG * (g + 1)], (1, 0, 2)).reshape(D, JW).astype(bf)
           for g in range(2)]
    wkg = [np.transpose(Wk[HG * g : HG * (g + 1)], (1, 0, 2)).reshape(D, JW).astype(bf)
           for g in range(2)]
    wvg = [np.transpose(Wv[HG * g : HG * (g + 1)], (1, 0, 2)).reshape(D, JW).astype(bf)
           for g in range(2)]
    wog = [Wo[JW * g : JW * (g + 1), :].astype(bf) for g in range(2)]

    in_maps = []
    for c in range(8):
        b, g = c // 2, c % 2
        in_maps.append({
            "xq": xqT[b], "xk": xkT[b], "xv": xvT[b], "mt": mtT[b],
            "wq": wqg[g], "wk": wkg[g], "wv": wvg[g], "wo": wog[g],
        })

    res = bass_utils.run_bass_kernel_spmd(nc, in_maps, core_ids=list(range(8)))
    outs = [r["out"] for r in res.results]
    return np.stack([outs[2 * b] + outs[2 * b + 1] for b in range(B)]).astype(np.float32)


# revision 32
# speedup vs baseline: 1.0497x; 1.0468x over previous
"""Multi-head attention kernel for 8 TRN2 NeuronCores.

Problem: B=4, S=2048, D=1024, H=16, DK=DV=64 multi-head attention with a
0/1 mask, f32 reference.

Sharding: 8 cores = 4 batches x 2 head-groups (8 heads each). Each core
computes, for its (batch, head-group): Q/K/V projections, masked softmax
attention, and a PARTIAL output projection (its heads' slice of Wo). The
two partials per batch are summed on the host (the tensor-parallel
all-reduce of the sharding hint, done host-side since full inputs/outputs
pass through the host anyway).

Device compute in bf16 with f32 PSUM accumulation:
 - Activations are pre-transposed on host: xq/xk/xv = x[b].T  [D, S].
 - Q^T,K^T computed as [j, s] (head pairs packed across 128 partitions);
   Q pre-scaled by 1/sqrt(DK).
 - Scores computed TRANSPOSED: S^T[t, s] = sum_j K^T[j,t] Q^T[j,s], so
   exp(S^T) directly feeds the attn@V matmul as the moving operand.
 - Softmax without max-subtraction (scores ~N(0,1); validated range).
   Mask applied multiplicatively after exp: P = exp(S^T) * maskT.
 - attn@V: lhsT = [V | ones] per head (65 cols) -> O^T rows 0..63 plus
   the softmax denominator (rowsum) in row 64, free on the PE.
 - t-blocks processed in pairs through a 2-bank PSUM tile (3 buffers):
   one exp and one mask multiply per pair, keeping PE runs uniform and
   per-op overheads amortized.
 - Normalization: rowsums DMA'd to partitions 0..7, reciprocal there,
   bounced through DRAM and broadcast across partitions by DMA; one
   elementwise multiply on the packed concat^T.
 - Output projection: packed head-pairs (k=128), result DMA'd straight
   from PSUM to DRAM.
"""

import numpy as np
import ml_dtypes
from contextlib import ExitStack

import concourse.bass as bass
import concourse.mybir as mybir
import concourse.tile as tile
from concourse import bacc
import concourse.bass_utils as bass_utils

P = 128
S = 2048          # sequence length
D = 1024          # model dim
HG = 8            # heads per core
DK = 64           # head dim
JW = HG * DK      # 512: packed projection width per core
DO = D // P       # 8 d-outer subtiles
NT = S // P       # 16 t-blocks
SC = 4            # s-chunks
SCW = S // SC     # 512 chunk width
NPAIR = HG // 2   # 4 head pairs
VW = DK + 1       # 65: V columns + ones column

bf16 = mybir.dt.bfloat16
f32 = mybir.dt.float32
AF = mybir.ActivationFunctionType
ALU = mybir.AluOpType


def _build():
    nc = bacc.Bacc("TRN2", target_bir_lowering=False, debug=False, num_devices=8)

    xq = nc.dram_tensor("xq", [D, S], bf16, kind="ExternalInput").ap()
    xk = nc.dram_tensor("xk", [D, S], bf16, kind="ExternalInput").ap()
    xv = nc.dram_tensor("xv", [D, S], bf16, kind="ExternalInput").ap()
    mt = nc.dram_tensor("mt", [S, S], bf16, kind="ExternalInput").ap()
    wq = nc.dram_tensor("wq", [D, JW], bf16, kind="ExternalInput").ap()
    wk = nc.dram_tensor("wk", [D, JW], bf16, kind="ExternalInput").ap()
    wv = nc.dram_tensor("wv", [D, JW], bf16, kind="ExternalInput").ap()
    wo = nc.dram_tensor("wo", [JW, D], bf16, kind="ExternalInput").ap()
    out = nc.dram_tensor("out", [S, D], f32, kind="ExternalOutput").ap()
    rscr = nc.dram_tensor("rscr", [SC, HG, SCW], bf16, kind="Internal").ap()
    rsum = nc.dram_tensor("rsum", [SC, HG, SCW], f32, kind="Internal").ap()

    with tile.TileContext(nc) as tc:
        with ExitStack() as ctx:
            consts = ctx.enter_context(tc.tile_pool(name="consts", bufs=1))
            stream = ctx.enter_context(tc.tile_pool(name="stream", bufs=8))
            mpool = ctx.enter_context(tc.tile_pool(name="mask", bufs=2))
            ppool = ctx.enter_context(tc.tile_pool(name="pp", bufs=6))
            epool = ctx.enter_context(tc.tile_pool(name="ep", bufs=4))
            rpool = ctx.enter_context(tc.tile_pool(name="rp", bufs=1))
            psA = ctx.enter_context(tc.tile_pool(name="psA", bufs=2, space="PSUM"))
            psO = ctx.enter_context(tc.tile_pool(name="psO", bufs=2, space="PSUM"))
            psP = ctx.enter_context(tc.tile_pool(name="psP", bufs=2, space="PSUM"))

            # ---- weights ----
            wq_sb = consts.tile([P, DO, JW], bf16, tag="wq")
            nc.sync.dma_start(wq_sb[:], wq.rearrange("(o p) j -> p o j", p=P))
            wk_sb = consts.tile([P, DO, JW], bf16, tag="wk")
            nc.sync.dma_start(wk_sb[:], wk.rearrange("(o p) j -> p o j", p=P))
            wv_sb = consts.tile([P, DO, JW], bf16, tag="wv")
            nc.sync.dma_start(wv_sb[:], wv.rearrange("(o p) j -> p o j", p=P))
            wo_sb = consts.tile([P, JW // P, D], bf16, tag="wo")
            nc.sync.dma_start(wo_sb[:], wo.rearrange("(o p) d -> p o d", p=P))

            # ---- persistent activations ----
            # QTz: per-head Q^T with the OTHER parity's 64 partitions zeroed, so
            # scores matmuls can contract over the full 128 partitions of KT
            # (uniform tile geometry on the PE; the off-parity K rows hit zeros).
            # V is padded so each head's attn@V lhsT slice [65h : 65h+128] is a
            # full 128 columns (FWL-eligible, uniform PE tile geometry); the
            # junk columns beyond [V | ones] write PSUM rows 65..127 which are
            # never read.
            VWID = VW * (HG - 1) + P  # 583
            QTz = consts.tile([P, HG, S], bf16, tag="qt")
            KT = consts.tile([P, NPAIR, S], bf16, tag="kt")
            V = consts.tile([P, NT, VWID], bf16, tag="v")  # [t_in, t_out, 65h + (v|ones)]
            CT = consts.tile([P, NPAIR, S], bf16, tag="ct")   # concat^T, normalized in place

            nc.vector.memset(QTz[:], 0.0)
            nc.vector.memset(V[:, :, HG * VW : VWID], 0.0)
            for h in range(HG):
                nc.vector.memset(V[:, :, h * VW + DK : h * VW + DK + 1], 1.0)

            # ---- Q/K projections: dst[jo*128+m, s] = sum_d w[d, jo*128+m] x[d, s]
            for xin, wsb, which, scale in ((xq, wq_sb, "q", 1.0 / 8.0), (xk, wk_sb, "k", 1.0)):
                for st in range(SC):
                    pa = psA.tile([P, 2, SCW], f32, tag="s2", name="pa")
                    pb0 = psP.tile([P, SCW], f32, tag="po", name="pb0")
                    pb1 = psP.tile([P, SCW], f32, tag="po", name="pb1")
                    tgts = [pa[:, 0, :], pa[:, 1, :], pb0[:], pb1[:]]
                    for do in range(DO):
                        xt = stream.tile([P, SCW], bf16, tag="xt")
                        dma_eng = nc.sync if do % 2 == 0 else nc.gpsimd
                        dma_eng.dma_start(
                            xt[:], xin[do * P : (do + 1) * P, st * SCW : (st + 1) * SCW]
                        )
                        for jo in range(NPAIR):
                            nc.tensor.matmul(
                                tgts[jo],
                                lhsT=wsb[:, do, jo * P : (jo + 1) * P],
                                rhs=xt[:],
                                start=(do == 0),
                                stop=(do == DO - 1),
                            )
                    ssl = slice(st * SCW, (st + 1) * SCW)
                    if which == "q":
                        for jo in range(NPAIR):
                            # scalar engine for one parity, vector for the other
                            nc.scalar.activation(
                                QTz[0:64, 2 * jo, ssl], tgts[jo][0:64, :],
                                AF.Copy, scale=scale,
                            )
                            nc.vector.tensor_scalar_mul(
                                QTz[64:128, 2 * jo + 1, ssl], tgts[jo][64:128, :], scale
                            )
                    else:
                        nc.scalar.activation(KT[:, 0:2, ssl], pa[:], AF.Copy, scale=scale)
                        nc.vector.tensor_copy(KT[:, 2:3, ssl], pb0[:, None, :])
                        nc.vector.tensor_copy(KT[:, 3:4, ssl], pb1[:, None, :])

            # ---- V projection: V[t, v] = sum_d x[d, t]^T w[d, v], strided into [V|ones] slots
            Vv = V[:, :, 0 : HG * VW].rearrange("p t (h c) -> p t h c", h=HG)
            for tbq in range(NT // 4):
                pva = psA.tile([P, 2, SCW], f32, tag="s2", name="pva")
                pvb0 = psP.tile([P, SCW], f32, tag="po", name="pvb0")
                pvb1 = psP.tile([P, SCW], f32, tag="po", name="pvb1")
                vtgts = [pva[:, 0, :], pva[:, 1, :], pvb0[:], pvb1[:]]
                for do in range(DO):
                    xt = stream.tile([P, SCW], bf16, tag="xt")
                    dma_eng = nc.sync if do % 2 == 0 else nc.gpsimd
                    dma_eng.dma_start(
                        xt[:], xv[do * P : (do + 1) * P, tbq * SCW : (tbq + 1) * SCW]
                    )
                    for i in range(4):
                        nc.tensor.matmul(
                            vtgts[i],
                            lhsT=xt[:, i * P : (i + 1) * P],
                            rhs=wv_sb[:, do, :],
                            start=(do == 0), stop=(do == DO - 1),
                        )
                for i in range(4):
                    eng = nc.vector.tensor_copy if i % 2 == 0 else nc.scalar.activation
                    src = vtgts[i].rearrange("p (h c) -> p h c", h=HG)
                    if i % 2 == 0:
                        nc.vector.tensor_copy(Vv[:, 4 * tbq + i, :, 0:DK], src)
                    else:
                        nc.scalar.activation(Vv[:, 4 * tbq + i, :, 0:DK], src, AF.Copy)

            # ---- attention ----
            mtr = mt.rearrange("(to p) s -> p to s", p=P)

            def load_mask(sc):
                mk = mpool.tile([P, NT, SCW], bf16, tag="mk", name=f"mk{sc}")
                nc.gpsimd.dma_start(mk[:], mtr[:, :, sc * SCW : (sc + 1) * SCW])
                return mk

            mk_tiles = {0: load_mask(0)}
            for sc in range(SC):
                mk = mk_tiles.pop(sc)
                if sc + 1 < SC:
                    mk_tiles[sc + 1] = load_mask(sc + 1)
                rs = rpool.tile([P, HG, SCW], f32, tag="rs")  # rowsum staging @ partition 64
                R2 = rpool.tile([P, SCW // 8], f32, tag="R2")  # rowsums spread over 64 partitions

                for h in range(HG):
                    o = h // 2
                    Ops = psO.tile([P, SCW], f32, tag="o", name="Ops")
                    for g in range(NT // 2):
                        S2 = psA.tile([P, 2, SCW], f32, tag="s2", name="S2")
                        for i in range(2):
                            tb = 2 * g + i
                            nc.tensor.matmul(
                                S2[:, i, :],
                                lhsT=KT[:, o, tb * P : (tb + 1) * P],
                                rhs=QTz[:, h, sc * SCW : (sc + 1) * SCW],
                                start=True, stop=True,
                            )
                        Pt = ppool.tile([P, 2, SCW], bf16, tag="pt")
                        nc.scalar.activation(Pt[:], S2[:], AF.Exp)
                        nc.vector.tensor_tensor(
                            Pt[:], Pt[:], mk[:, 2 * g : 2 * g + 2, :], ALU.mult
                        )
                        for i in range(2):
                            tb = 2 * g + i
                            nc.tensor.matmul(
                                Ops[:],
                                lhsT=V[:, tb, h * VW : h * VW + P],
                                rhs=Pt[:, i, :],
                                start=(tb == 0), stop=(tb == NT - 1),
                            )
                    # evacuate rowsum + O^T
                    nc.vector.tensor_copy(rs[64:65, h, :], Ops[64:65, :])
                    if h % 2 == 0:
                        nc.vector.tensor_copy(
                            CT[0:64, o, sc * SCW : (sc + 1) * SCW], Ops[0:64, :]
                        )
                    else:
                        ob = epool.tile([64, SCW], bf16, tag="ob")
                        nc.vector.tensor_copy(ob[:], Ops[0:64, :])
                        nc.gpsimd.dma_start(
                            CT[64:128, o, sc * SCW : (sc + 1) * SCW], ob[:]
                        )

                # ---- normalization factors ----
                # bounce rowsums through DRAM to spread them over 64 partitions
                nc.sync.dma_start(rsum[sc].unsqueeze(0), rs[64:65, :, :])
                nc.sync.dma_start(
                    R2[0:64, :], rsum[sc].rearrange("h (p s) -> (h p) s", p=8)
                )
                Rr = rpool.tile([P, SCW // 8], bf16, tag="Rr")
                with nc.allow_low_precision(reason="softmax denominators tolerate bf16"):
                    nc.vector.reciprocal(Rr[0:64, :], R2[0:64, :])
                nc.sync.dma_start(
                    rscr[sc].rearrange("h (p s) -> (h p) s", p=8), Rr[0:64, :]
                )
                Rf = rpool.tile([P, NPAIR, SCW], bf16, tag="Rf")
                for par in range(2):
                    eng = nc.sync if par == 0 else nc.gpsimd
                    eng.dma_start(
                        Rf[64 * par : 64 * par + 64, :, :],
                        rscr[sc].rearrange("(pr two) s -> two pr s", two=2)[par]
                        .unsqueeze(0).to_broadcast([64, NPAIR, SCW]),
                    )

                # ---- normalize + output projection, per s-block so the PE can
                # start projecting before the whole chunk is normalized ----
                for sb in range(SCW // P):
                    s0 = sc * SCW + sb * P
                    nc.vector.tensor_tensor(
                        CT[:, :, s0 : s0 + P],
                        CT[:, :, s0 : s0 + P],
                        Rf[:, :, sb * P : (sb + 1) * P], ALU.mult,
                    )
                    for dt in range(D // SCW):
                        po = psP.tile([P, SCW], f32, tag="po", name="po")
                        for o2 in range(NPAIR):
                            nc.tensor.matmul(
                                po[:],
                                lhsT=CT[:, o2, s0 : s0 + P],
                                rhs=wo_sb[:, o2, dt * SCW : (dt + 1) * SCW],
                                start=(o2 == 0), stop=(o2 == NPAIR - 1),
                            )
                        ot = epool.tile([P, SCW], f32, tag="ot")
                        nc.vector.tensor_copy(ot[:], po[:])
                        nc.sync.dma_start(
                            out[s0 : s0 + P, dt * SCW : (dt + 1) * SCW], ot[:]
                        )

    nc.compile()
    return nc


_NC = None


def _get_nc():
    global _NC
    if _NC is None:
        _NC = _build()
    return _NC


def kernel(queries, keys, values, mask, Wq, Wk, Wv, Wo):
    bf = ml_dtypes.bfloat16
    B = queries.shape[0]
    nc = _get_nc()

    xqT = [queries[b].T.astype(bf) for b in range(B)]
    xkT = [keys[b].T.astype(bf) for b in range(B)]
    xvT = [values[b].T.astype(bf) for b in range(B)]
    mtT = [(mask[b] != 0).T.astype(bf) for b in range(B)]
    wqg = [np.transpose(Wq[HG * g : HG * (g + 1)], (1, 0, 2)).reshape(D, JW).astype(bf)
           for g in range(2)]
    wkg = [np.transpose(Wk[HG * g : HG * (g + 1)], (1, 0, 2)).reshape(D, JW).astype(bf)
           for g in range(2)]
    wvg = [np.transpose(Wv[HG * g : HG * (g + 1)], (1, 0, 2)).reshape(D, JW).astype(bf)
           for g in range(2)]
    wog = [Wo[JW * g : JW * (g + 1), :].astype(bf) for g in range(2)]

    in_maps = []
    for c in range(8):
        b, g = c // 2, c % 2
        in_maps.append({
            "xq": xqT[b], "xk": xkT[b], "xv": xvT[b], "mt": mtT[b],
            "wq": wqg[g], "wk": wkg[g], "wv": wvg[g], "wo": wog[g],
        })

    res = bass_utils.run_bass_kernel_spmd(nc, in_maps, core_ids=list(range(8)))
    outs = [r["out"] for r in res.results]
    return np.stack([outs[2 * b] + outs[2 * b + 1] for b in range(B)]).astype(np.float32)
